# revision 20
# baseline (speedup 1.0000x reference)
"""Bidirectional Mamba encoder on 8 Trainium2 NeuronCores.

Sharding: 4 streams (batch 2 x directions fwd/bwd) x tensor-parallel 2
(d_inner and ffn_dim split in half) = 8 cores; core 2s+p = stream s, half p.
The xm path (in_proj xm part / conv / x_proj) is computed fully inside each
pair so the only cross-core collectives are the out_proj and ffn2
partial-sum AllReduces (the final layer's ffn2 reduction is deferred to the
host gather).

Layout: feature-major [feature_partition, token_free] everywhere.  The
selective scan uses the HW prefix scan (tensor_tensor_scan), one [128, T]
scan per (e-tile, state-n).  softplus(x) is computed as -ln(sigmoid(-x));
the sign is absorbed into the exp scale (negA) and one tensor_scalar negate.
"""

import numpy as np

import concourse.bass as bass
import concourse.bacc as bacc
import concourse.mybir as mybir
from concourse import tile
from concourse.bass_utils import run_bass_kernel_spmd

F32 = mybir.dt.float32
BF16 = mybir.dt.bfloat16
AF = mybir.ActivationFunctionType
OP = mybir.AluOpType

D_MODEL = 512
N_LAYERS = 2
FFN_DIM = 896
D_STATE = 16
D_CONV = 4
D_INNER = 1024
DT_RANK = 32
B, T = 2, 512

HALF_E = D_INNER // 2
HALF_F = FFN_DIM // 2
HALF_F_PAD = 512
NE = D_INNER // 128          # 8 full e-tiles
NEH = HALF_E // 128          # 4 own e-tiles
ND = D_MODEL // 128          # 4 d-tiles
EPS = 1e-5
RG = [[0, 1], [2, 3], [4, 5], [6, 7]]

_CACHED = {}


def _build():
    nc = bacc.Bacc("TRN2", target_bir_lowering=False, debug=False,
                   num_devices=8)
    P = {}

    def dparam(name, shape, dtype=F32):
        P[name] = nc.declare_dram_parameter(name, list(shape), dtype,
                                            isOutput=False)

    dparam("xin", [ND, 128, T])
    dparam("mask", [1, T])
    dparam("statw", [128, 1], BF16)
    dparam("ones_row", [1, 128], BF16)
    dparam("ident", [128, 128], BF16)
    dparam("gate", [128, 1])
    dparam("ln0w", [128, ND]); dparam("ln0b", [128, ND])
    for l in range(N_LAYERS):
        dparam(f"w_in{l}", [ND, 128, 1536], BF16)
        dparam(f"cw{l}", [128, NE * D_CONV])
        dparam(f"convb{l}", [128, NE])
        dparam(f"w_xp{l}", [NE, 128, 64], BF16)
        dparam(f"w_dt{l}", [32, HALF_E], BF16)
        dparam(f"ndtb{l}", [128, NEH])                # NEGATED dt_proj_b
        dparam(f"negA{l}", [128, NEH * D_STATE])      # -A[e,n] (positive)
        dparam(f"Ddiag{l}", [NEH, 128, 128], BF16)
        dparam(f"w_out{l}", [NEH, 128, D_MODEL], BF16)
        dparam(f"w_f1{l}", [ND, 128, HALF_F_PAD], BF16)
        dparam(f"b1{l}", [128, ND])
        dparam(f"w_f2{l}", [ND, 128, D_MODEL], BF16)
        dparam(f"b2g{l}", [128, ND])
        for ln in ("ln1", "ln2"):
            dparam(f"{ln}w{l}", [128, ND])
            dparam(f"{ln}b{l}", [128, ND])
    out_ext = nc.declare_dram_parameter("out", [ND, 128, T], F32,
                                        isOutput=True)

    with tile.TileContext(nc) as tc:
        with (
            tc.tile_pool(name="wc", bufs=1) as wc,        # constants
            tc.tile_pool(name="wl", bufs=1) as wl,
            tc.tile_pool(name="wcv", bufs=8) as wcv,        # per-layer weights
            tc.tile_pool(name="wi", bufs=1) as wi,        # in_proj (prefetch)
            tc.tile_pool(name="act", bufs=1) as act,
            tc.tile_pool(name="sm", bufs=1) as sm,        # small [1,T] rows
            tc.tile_pool(name="scan", bufs=4) as scanp,
            tc.tile_pool(name="bc", bufs=1) as bcp,
            tc.tile_pool(name="pp", bufs=3, space="PSUM") as pp,
            tc.tile_pool(name="pd", bufs=1, space="PSUM") as pd,
            tc.tile_pool(name="py", bufs=2, space="PSUM") as py,
            tc.tile_pool(name="pst", bufs=1, space="PSUM") as pst,
            tc.tile_pool(name="dram", bufs=2, space="DRAM") as dram,
        ):
            # ---- constants ----
            def cload(name, pool=wc, tag=None):
                shp = list(P[name].shape)
                t_ = pool.tile(shp, P[name].dtype, tag=tag or name)
                nc.sync.dma_start(t_[:], P[name][:])
                return t_

            # critical-path loads first: residual input + LN0 params
            x_cur = []
            for j in range(ND):
                xt = act.tile([128, T], F32, tag=f"tmp{j}", name=f"xin{j}")
                nc.sync.dma_start(xt[:], P["xin"][j])
                x_cur.append(xt)
            statw = cload("statw")
            ones_row = cload("ones_row")
            ident = cload("ident")
            gate = cload("gate")
            ln0w = cload("ln0w"); ln0b = cload("ln0b")
            lnp = {}
            for l in range(N_LAYERS):
                for ln in ("ln1", "ln2"):
                    lnp[f"{ln}w{l}"] = cload(f"{ln}w{l}")
                    lnp[f"{ln}b{l}"] = cload(f"{ln}b{l}")
            smallp = {}
            for l in range(N_LAYERS):
                for nm in (f"convb{l}", f"ndtb{l}", f"negA{l}", f"b1{l}",
                           f"b2g{l}", f"cw{l}"):
                    smallp[nm] = cload(nm)

            maskrep = wc.tile([128, T], F32, tag="maskrep")
            nc.sync.dma_start(
                maskrep[:], P["mask"][:].partition_broadcast(128)[:, 0, :])

            xm_pad = nc.alloc_sbuf_tensor("xm_pad", [128, NE * (T + 3)], BF16)
            nc.vector.memset(xm_pad.ap()[:], 0.0)

            def layernorm(x_tiles, w_col, b_col, out_dtype, out_tags):
                xb, x2 = [], []
                for j in range(ND):
                    b_ = act.tile([128, T], BF16, tag=f"lnxb{j}")
                    s_ = act.tile([128, T], BF16, tag=f"lnx2{j}")
                    nc.vector.tensor_copy(b_[:], x_tiles[j][:])
                    nc.scalar.activation(s_[:], x_tiles[j][:], AF.Square)
                    xb.append(b_); x2.append(s_)
                mu_ps = pst.tile([1, T], F32, tag="mu")
                e2_ps = pst.tile([1, T], F32, tag="e2")
                for j in range(ND):
                    nc.tensor.matmul(mu_ps[:], statw[:, 0:1], xb[j][:],
                                     start=(j == 0), stop=(j == ND - 1))
                for j in range(ND):
                    nc.tensor.matmul(e2_ps[:], statw[:, 0:1], x2[j][:],
                                     start=(j == 0), stop=(j == ND - 1))
                mus = sm.tile([1, T], F32, tag="mus")
                mub = sm.tile([1, T], BF16, tag="mub")
                e2s = sm.tile([1, T], F32, tag="e2s")
                nc.scalar.activation(mus[:], mu_ps[:], AF.Copy)
                nc.scalar.activation(mub[:], mu_ps[:], AF.Copy)
                nc.scalar.activation(e2s[:], e2_ps[:], AF.Copy)
                mu2 = sm.tile([1, T], F32, tag="mu2")
                var = sm.tile([1, T], F32, tag="var")
                sd = sm.tile([1, T], F32, tag="sd")
                rb = sm.tile([1, T], BF16, tag="rb")
                nc.vector.tensor_tensor(mu2[:], mus[:], mus[:], OP.mult)
                nc.vector.tensor_tensor(var[:], e2s[:], mu2[:], OP.subtract)
                nc.vector.tensor_scalar_add(var[:], var[:], EPS)
                nc.scalar.activation(sd[:], var[:], AF.Sqrt)
                with nc.allow_low_precision(reason="bf16 rstd is enough"):
                    nc.vector.reciprocal(rb[:], sd[:])
                murep = act.tile([128, T], BF16, tag="murep")
                rrep = act.tile([128, T], BF16, tag="rrep")
                mu_rep_ps = pp.tile([128, T], F32, tag="mm")
                nc.tensor.matmul(mu_rep_ps[:], ones_row[:], mub[:],
                                 start=True, stop=True)
                nc.scalar.activation(murep[:], mu_rep_ps[:], AF.Copy)
                r_rep_ps = pp.tile([128, T], F32, tag="mm")
                nc.tensor.matmul(r_rep_ps[:], ones_row[:], rb[:],
                                 start=True, stop=True)
                nc.scalar.activation(rrep[:], r_rep_ps[:], AF.Copy)
                outs = []
                for j in range(ND):
                    xh = act.tile([128, T], BF16, tag=f"lnxh{j}")
                    nc.vector.tensor_tensor(xh[:], xb[j][:], murep[:],
                                            OP.subtract)
                    nc.vector.tensor_tensor(xh[:], xh[:], rrep[:], OP.mult)
                    o = act.tile([128, T], out_dtype, tag=out_tags.format(j=j))
                    nc.scalar.activation(o[:], xh[:], AF.Identity,
                                         scale=w_col[:, j:j + 1],
                                         bias=b_col[:, j:j + 1])
                    outs.append(o)
                return outs

            x_cur = layernorm(x_cur, ln0w, ln0b, F32, "xr{j}")

            for l in range(N_LAYERS):
                # per-layer weight loads (shared tags -> reused slots)
                w_in = []
                for k in range(ND):
                    t_ = wi.tile([128, 1536], BF16, tag=f"w_in{k}")
                    nc.sync.dma_start(t_[:], P[f"w_in{l}"][k])
                    w_in.append(t_)

                w_xp = []
                for k in range(NE):
                    t_ = wl.tile([128, 64], BF16, tag=f"w_xp{k}")
                    nc.sync.dma_start(t_[:], P[f"w_xp{l}"][k])
                    w_xp.append(t_)
                w_dt = wl.tile([32, HALF_E], BF16, tag="w_dt")
                nc.sync.dma_start(w_dt[:], P[f"w_dt{l}"][:])
                Ddiag = []
                for k in range(NEH):
                    t_ = wl.tile([128, 128], BF16, tag=f"Ddiag{k}")
                    nc.sync.dma_start(t_[:], P[f"Ddiag{l}"][k])
                    Ddiag.append(t_)
                w_out = []
                for k in range(NEH):
                    t_ = wl.tile([128, D_MODEL], BF16, tag=f"w_out{k}")
                    nc.sync.dma_start(t_[:], P[f"w_out{l}"][k])
                    w_out.append(t_)
                w_f1 = []
                for k in range(ND):
                    t_ = wl.tile([128, HALF_F_PAD], BF16, tag=f"w_f1{k}")
                    nc.sync.dma_start(t_[:], P[f"w_f1{l}"][k])
                    w_f1.append(t_)
                w_f2 = []
                for k in range(ND):
                    t_ = wl.tile([128, D_MODEL], BF16, tag=f"w_f2{k}")
                    nc.sync.dma_start(t_[:], P[f"w_f2{l}"][k])
                    w_f2.append(t_)

                xh = layernorm(x_cur, lnp[f"ln1w{l}"], lnp[f"ln1b{l}"], BF16,
                               "lno{j}")

                # ---- in_proj ----
                zs = []
                for m in range(12):
                    ps = pp.tile([128, T], F32, tag="mm")
                    for k in range(ND):
                        nc.tensor.matmul(ps[:],
                                         w_in[k][:, m * 128:(m + 1) * 128],
                                         xh[k][:], start=(k == 0),
                                         stop=(k == ND - 1))
                    if m < 8:
                        off = m * (T + 3) + 3
                        nc.scalar.activation(xm_pad.ap()[:, off:off + T],
                                             ps[:], AF.Copy)
                    else:
                        z = act.tile([128, T], BF16, tag=f"z{m - 8}")
                        nc.scalar.activation(z[:], ps[:], AF.Silu)
                        zs.append(z)

                # ---- conv (DVE taps) + silu -> u ----
                u = []
                for j in range(NE):
                    base = j * (T + 3)
                    cacc = act.tile([128, T], BF16, tag="cacc")
                    ct = [scanp.tile([128, T], BF16, tag="ctap",
                                     name=f"ct{k}") for k in range(D_CONV)]
                    for k in range(D_CONV):
                        nc.vector.tensor_scalar_mul(
                            ct[k][:], xm_pad.ap()[:, base + k:base + k + T],
                            smallp[f"cw{l}"][:, j * D_CONV + k:
                                             j * D_CONV + k + 1])
                    nc.vector.tensor_tensor(cacc[:], ct[0][:], ct[1][:],
                                            OP.add)
                    nc.vector.tensor_tensor(cacc[:], cacc[:], ct[2][:],
                                            OP.add)
                    nc.vector.tensor_tensor(cacc[:], cacc[:], ct[3][:],
                                            OP.add)
                    uj = act.tile([128, T], BF16, tag=f"u{j}")
                    nc.scalar.activation(uj[:], cacc[:], AF.Silu,
                                         bias=smallp[f"convb{l}"][:, j:j + 1])
                    u.append(uj)

                # ---- x_proj ----
                dbl_ps = pd.tile([64, T], F32, tag="dbl")
                for k in range(NE):
                    nc.tensor.matmul(dbl_ps[:], w_xp[k][:], u[k][:],
                                     start=(k == 0), stop=(k == NE - 1))
                dbl = act.tile([64, T], BF16, tag="dbl_sb")
                nc.scalar.activation(dbl[:], dbl_ps[:], AF.Copy)

                # ---- B / C replication ----
                bcd = dram.tile([32, T], BF16, tag="bcd")
                nc.sync.dma_start(bcd[:], dbl[32:64, :])
                Brep, Crep = [], []
                for n in range(D_STATE):
                    br = bcp.tile([128, T], BF16, tag=f"Brep{n}")
                    cr = bcp.tile([128, T], BF16, tag=f"Crep{n}")
                    nc.sync.dma_start(
                        br[:], bcd[n:n + 1, :]
                        .partition_broadcast(128)[:, 0, :])
                    nc.sync.dma_start(
                        cr[:], bcd[16 + n:17 + n, :]
                        .partition_broadcast(128)[:, 0, :])
                    Brep.append(br); Crep.append(cr)

                # ---- delta for all m (batched per ACT table) ----
                sgs, dms, wdus = [], [], []
                for m in range(NEH):
                    ps = pp.tile([128, T], F32, tag="mm")
                    nc.tensor.matmul(ps[:],
                                     w_dt[0:32, m * 128:(m + 1) * 128],
                                     dbl[0:32, :], start=True, stop=True)
                    sg = act.tile([128, T], BF16, tag=f"sg{m}")
                    nc.scalar.activation(sg[:], ps[:], AF.Sigmoid, scale=-1.0,
                                         bias=smallp[f"ndtb{l}"][:, m:m + 1])
                    sgs.append(sg)
                for m in range(NEH):
                    dm = act.tile([128, T], BF16, tag=f"dm{m}")
                    nc.scalar.activation(dm[:], sgs[m][:], AF.Ln)
                    dms.append(dm)
                    # wdu = delta' * u = -softplus*u; sign fixed via negated
                    # C rows (host-side) since y is linear in dBu.
                    wdu = act.tile([128, T], BF16, tag=f"wdu{m}")
                    nc.vector.tensor_tensor(wdu[:], dm[:], u[m][:], OP.mult)
                    wdus.append(wdu)

                # warm up ncfw so the out_proj AllReduce pays less first-call
                # latency: tiny dummy collective issued early in the scan
                warm_in = dram.tile([1, 4], F32, tag="warm_in")
                warm_out = dram.tile([1, 4], F32, tag="warm_out")
                nc.sync.dma_start(warm_in[:], maskrep[0:1, 0:4])
                nc.gpsimd.collective_compute(
                    "AllReduce", OP.add, replica_groups=RG,
                    ins=[warm_in[:].opt()], outs=[warm_out[:].opt()])

                # ---- scan (per own e-tile) ----
                y_gated = []
                for m in range(NEH):
                    dm = dms[m]
                    wdu = wdus[m]
                    ps_y = py.tile([128, T], F32, tag="ps_y")
                    for n in range(D_STATE):
                        dA = scanp.tile([128, T], BF16, tag="dA")
                        nc.scalar.activation(
                            dA[:], dm[:], AF.Exp,
                            scale=smallp[f"negA{l}"]
                            [:, m * D_STATE + n: m * D_STATE + n + 1])
                        dBu = scanp.tile([128, T], BF16, tag="dBu")
                        nc.vector.tensor_tensor(dBu[:], wdu[:], Brep[n][:],
                                                OP.mult)
                        hs = scanp.tile([128, T], BF16, tag="hs")
                        nc.vector.tensor_tensor_scan(hs[:], dA[:], dBu[:],
                                                     0.0, OP.mult, OP.add)
                        hsC = scanp.tile([128, T], BF16, tag="hsC")
                        nc.vector.tensor_tensor(hsC[:], hs[:], Crep[n][:],
                                                OP.mult)
                        nc.tensor.matmul(ps_y[:], ident[:], hsC[:],
                                         start=(n == 0), stop=False)
                    nc.tensor.matmul(ps_y[:], Ddiag[m][:], u[m][:],
                                     start=False, stop=True)
                    yg = act.tile([128, T], BF16, tag=f"yg{m}")
                    nc.vector.tensor_tensor(yg[:], ps_y[:], zs[m][:], OP.mult)
                    y_gated.append(yg)

                # ---- out_proj partial + AllReduce ----
                arA_in = dram.tile([128, ND * T], BF16, tag="arA_in")
                arA_out = dram.tile([128, ND * T], BF16, tag="arA_out")
                poA = act.tile([128, ND * T], BF16, tag="stageA")
                for g in range(2):
                    pso = [pp.tile([128, T], F32, tag="mm", name=f"pso{j}")
                           for j in (2 * g, 2 * g + 1)]
                    for k in range(NEH):
                        for i, j in enumerate((2 * g, 2 * g + 1)):
                            nc.tensor.matmul(
                                pso[i][:], w_out[k][:, j * 128:(j + 1) * 128],
                                y_gated[k][:], start=(k == 0),
                                stop=(k == NEH - 1))
                    for i, j in enumerate((2 * g, 2 * g + 1)):
                        nc.scalar.activation(poA[:, j * T:(j + 1) * T],
                                             pso[i][:], AF.Copy)
                nc.sync.dma_start(arA_in[:], poA[:])
                nc.gpsimd.collective_compute(
                    "AllReduce", OP.add, replica_groups=RG,
                    ins=[arA_in[:].opt()], outs=[arA_out[:].opt()])
                moA = act.tile([128, ND * T], BF16, tag="stageAo")
                nc.sync.dma_start(moA[:], arA_out[:])
                x_mid = []
                for j in range(ND):
                    xm_ = act.tile([128, T], F32, tag=f"xmid{j}")
                    nc.vector.tensor_tensor(xm_[:], x_cur[j][:],
                                            moA[:, j * T:(j + 1) * T], OP.add)
                    x_mid.append(xm_)

                # ---- FFN ----
                xh2 = layernorm(x_mid, lnp[f"ln2w{l}"], lnp[f"ln2b{l}"], BF16,
                                "lno{j}")
                h = []
                for m in range(ND):
                    ps = pp.tile([128, T], F32, tag="mm")
                    for k in range(ND):
                        nc.tensor.matmul(ps[:],
                                         w_f1[k][:, m * 128:(m + 1) * 128],
                                         xh2[k][:], start=(k == 0),
                                         stop=(k == ND - 1))
                    hm = act.tile([128, T], BF16, tag=f"h{m}")
                    nc.scalar.activation(hm[:], ps[:], AF.Gelu,
                                         bias=smallp[f"b1{l}"][:, m:m + 1])
                    h.append(hm)

                if l < N_LAYERS - 1:
                    arB_in = dram.tile([128, ND * T], BF16, tag="arB_in")
                    arB_out = dram.tile([128, ND * T], BF16, tag="arB_out")
                    poB = act.tile([128, ND * T], BF16, tag="stageA")
                    for g in range(2):
                        psf = [pp.tile([128, T], F32, tag="mm",
                                       name=f"psf{j}")
                               for j in (2 * g, 2 * g + 1)]
                        for k in range(ND):
                            for i, j in enumerate((2 * g, 2 * g + 1)):
                                nc.tensor.matmul(
                                    psf[i][:],
                                    w_f2[k][:, j * 128:(j + 1) * 128],
                                    h[k][:], start=(k == 0),
                                    stop=(k == ND - 1))
                        for i, j in enumerate((2 * g, 2 * g + 1)):
                            nc.scalar.activation(
                                poB[:, j * T:(j + 1) * T], psf[i][:], AF.Copy)
                    nc.sync.dma_start(arB_in[:], poB[:])
                    nc.gpsimd.collective_compute(
                        "AllReduce", OP.add, replica_groups=RG,
                        ins=[arB_in[:].opt()], outs=[arB_out[:].opt()])
                    moB = act.tile([128, ND * T], BF16, tag="stageAo")
                    nc.sync.dma_start(moB[:], arB_out[:])
                    x_next = []
                    for j in range(ND):
                        t1 = act.tile([128, T], F32, tag=f"tmp{j}")
                        nc.vector.scalar_tensor_tensor(
                            t1[:], moB[:, j * T:(j + 1) * T],
                            smallp[f"b2g{l}"][:, j:j + 1],
                            x_mid[j][:], OP.add, OP.add)
                        xn = act.tile([128, T], F32, tag=f"xr{j}")
                        nc.vector.tensor_tensor(xn[:], t1[:], maskrep[:],
                                                OP.mult)
                        x_next.append(xn)
                    x_cur = x_next
                else:
                    for j in range(ND):
                        ps = pp.tile([128, T], F32, tag="mm")
                        for k in range(ND):
                            nc.tensor.matmul(
                                ps[:], w_f2[k][:, j * 128:(j + 1) * 128],
                                h[k][:], start=(k == 0), stop=(k == ND - 1))
                        t1 = act.tile([128, T], F32, tag=f"tmp{j}")
                        nc.vector.scalar_tensor_tensor(
                            t1[:], x_mid[j][:], gate[:, 0:1], ps[:],
                            OP.mult, OP.add)
                        t2 = act.tile([128, T], F32, tag=f"cpy{j}")
                        nc.vector.scalar_tensor_tensor(
                            t2[:], t1[:], smallp[f"b2g{l}"][:, j:j + 1],
                            maskrep[:], OP.add, OP.mult)
                        nc.sync.dma_start(out_ext[j], t2[:])

    nc.compile()
    return nc


def _sinusoidal_pe(Tn, D):
    pos = np.arange(Tn, dtype=np.float32)[:, None]
    div = np.exp(np.arange(0, D, 2, dtype=np.float32) *
                 (-np.log(10000.0) / D))
    pe = np.zeros((Tn, D), dtype=np.float32)
    pe[:, 0::2] = np.sin(pos * div)
    pe[:, 1::2] = np.cos(pos * div)
    return pe


def _col128(v, ncols):
    v = np.asarray(v, np.float32)
    if v.shape[0] < ncols * 128:
        v = np.pad(v, (0, ncols * 128 - v.shape[0]))
    return v.reshape(ncols, 128).T.copy()


def _prep_core_inputs(x, lengths, params):
    import ml_dtypes
    x = np.asarray(x, np.float32)
    lengths = np.asarray(lengths)
    pe = _sinusoidal_pe(T, D_MODEL)

    def f(a):
        return np.ascontiguousarray(np.asarray(a, np.float32))

    def bf(a):
        return np.ascontiguousarray(
            np.asarray(a, np.float32).astype(ml_dtypes.bfloat16))

    in_maps = []
    for c in range(8):
        s, p = c // 2, c % 2
        b, d = s // 2, s % 2
        prm = [params["fwd"], params["bwd"]][d]
        L = int(lengths[b])

        xi = (x[b] + pe).astype(np.float32)
        if d == 1:
            xi = xi[::-1]
        mask = (np.arange(T) < L).astype(np.float32)
        if d == 1:
            mask = mask[::-1]

        e_own = np.arange(p * HALF_E, (p + 1) * HALF_E)
        e_oth = np.arange((1 - p) * HALF_E, (2 - p) * HALF_E)
        e_ord = np.concatenate([e_own, e_oth])
        f_own = np.arange(p * HALF_F, (p + 1) * HALF_F)

        m = {
            "xin": f(xi.T.reshape(ND, 128, T)),
            "mask": f(mask[None, :]),
            "statw": bf(np.full((128, 1), 1.0 / D_MODEL)),
            "ones_row": bf(np.ones((1, 128))),
            "ident": bf(np.eye(128)),
            "gate": f(np.full((128, 1), 1.0 if p == 0 else 0.0)),
            "ln0w": f(_col128(prm[0]["ln0_w"], ND)),
            "ln0b": f(_col128(prm[0]["ln0_b"], ND)),
        }
        for l in range(N_LAYERS):
            lp = prm[l]
            ip = np.asarray(lp["in_proj_w"], np.float32)
            w_in = np.concatenate([ip[e_ord], ip[D_INNER + e_own]], 0).T
            m[f"w_in{l}"] = bf(np.ascontiguousarray(w_in).reshape(ND, 128,
                                                                  1536))
            cw = np.asarray(lp["conv_w"], np.float32)[e_ord]
            cwcols = np.zeros((128, NE * D_CONV), np.float32)
            for j in range(NE):
                for k in range(D_CONV):
                    cwcols[:, j * D_CONV + k] = cw[j * 128:(j + 1) * 128, k]
            m[f"cw{l}"] = f(cwcols)
            m[f"convb{l}"] = f(_col128(
                np.asarray(lp["conv_b"], np.float32)[e_ord], NE))
            xp = np.asarray(lp["x_proj_w"], np.float32).copy()
            xp[DT_RANK + D_STATE:] *= -1.0    # negate C rows (sign fold)
            m[f"w_xp{l}"] = bf(np.ascontiguousarray(xp[:, e_ord].T)
                               .reshape(NE, 128, 64))
            dtw = np.asarray(lp["dt_proj_w"], np.float32)
            m[f"w_dt{l}"] = bf(dtw[e_own].T)
            m[f"ndtb{l}"] = f(_col128(
                -np.asarray(lp["dt_proj_b"], np.float32)[e_own], NEH))
            A = -np.exp(np.asarray(lp["A_log"], np.float32))
            negA = -A[e_own]
            m[f"negA{l}"] = f(np.concatenate(
                [negA[mm * 128:(mm + 1) * 128] for mm in range(NEH)], axis=1))
            Dv = np.asarray(lp["D"], np.float32)[e_own]
            dd = np.zeros((NEH, 128, 128), np.float32)
            for mm in range(NEH):
                np.fill_diagonal(dd[mm], Dv[mm * 128:(mm + 1) * 128])
            m[f"Ddiag{l}"] = bf(dd)
            ow = np.asarray(lp["out_proj_w"], np.float32)
            m[f"w_out{l}"] = bf(np.ascontiguousarray(ow[:, e_own].T)
                                .reshape(NEH, 128, D_MODEL))
            f1 = np.asarray(lp["ffn_w1"], np.float32)
            f1p = np.zeros((HALF_F_PAD, D_MODEL), np.float32)
            f1p[:HALF_F] = f1[f_own]
            m[f"w_f1{l}"] = bf(np.ascontiguousarray(f1p.T)
                               .reshape(ND, 128, HALF_F_PAD))
            b1p = np.zeros(HALF_F_PAD, np.float32)
            b1p[:HALF_F] = np.asarray(lp["ffn_b1"], np.float32)[f_own]
            m[f"b1{l}"] = f(_col128(b1p, ND))
            f2 = np.asarray(lp["ffn_w2"], np.float32)
            f2p = np.zeros((HALF_F_PAD, D_MODEL), np.float32)
            f2p[:HALF_F] = f2[:, f_own].T
            m[f"w_f2{l}"] = bf(f2p.reshape(ND, 128, D_MODEL))
            b2 = np.asarray(lp["ffn_b2"], np.float32)
            m[f"b2g{l}"] = f(_col128(b2 if p == 0 else np.zeros_like(b2), ND))
            for ln, wkey, bkey in (("ln1", "ln1_w", "ln1_b"),
                                   ("ln2", "ln2_w", "ln2_b")):
                m[f"{ln}w{l}"] = f(_col128(lp[wkey], ND))
                m[f"{ln}b{l}"] = f(_col128(lp[bkey], ND))
        in_maps.append(m)
    return in_maps


def kernel(x, lengths, params):
    if "nc" not in _CACHED:
        _CACHED["nc"] = _build()
    nc = _CACHED["nc"]
    in_maps = _prep_core_inputs(x, lengths, params)
    res = run_bass_kernel_spmd(nc, in_maps, list(range(8)))
    outs = [np.asarray(res.results[c]["out"], np.float32)
            .reshape(D_MODEL, T) for c in range(8)]
    final = np.zeros((B, T, D_MODEL), np.float32)
    for b in range(B):
        fwd = (outs[4 * b + 0] + outs[4 * b + 1]).T
        bwd = (outs[4 * b + 2] + outs[4 * b + 3]).T[::-1]
        final[b] = fwd + bwd
    return final


# revision 21
# speedup vs baseline: 1.0244x; 1.0244x over previous
"""Bidirectional Mamba encoder on 8 Trainium2 NeuronCores.

Sharding: 4 streams (batch 2 x directions fwd/bwd) x tensor-parallel 2
(d_inner and ffn_dim split in half) = 8 cores; core 2s+p = stream s, half p.
The xm path (in_proj xm part / conv / x_proj) is computed fully inside each
pair so the only cross-core collectives are the out_proj and ffn2
partial-sum AllReduces (the final layer's ffn2 reduction is deferred to the
host gather).

Layout: feature-major [feature_partition, token_free] everywhere.  The
selective scan uses the HW prefix scan (tensor_tensor_scan), one [128, T]
scan per (e-tile, state-n).  softplus(x) is computed as -ln(sigmoid(-x));
the sign is absorbed into the exp scale (negA) and one tensor_scalar negate.
"""

import numpy as np

import concourse.bass as bass
import concourse.bacc as bacc
import concourse.mybir as mybir
from concourse import tile
from concourse.bass_utils import run_bass_kernel_spmd

F32 = mybir.dt.float32
BF16 = mybir.dt.bfloat16
AF = mybir.ActivationFunctionType
OP = mybir.AluOpType

D_MODEL = 512
N_LAYERS = 2
FFN_DIM = 896
D_STATE = 16
D_CONV = 4
D_INNER = 1024
DT_RANK = 32
B, T = 2, 512

HALF_E = D_INNER // 2
HALF_F = FFN_DIM // 2
HALF_F_PAD = 512
NE = D_INNER // 128          # 8 full e-tiles
NEH = HALF_E // 128          # 4 own e-tiles
ND = D_MODEL // 128          # 4 d-tiles
EPS = 1e-5
RG = [[0, 1], [2, 3], [4, 5], [6, 7]]

_CACHED = {}


def _build():
    nc = bacc.Bacc("TRN2", target_bir_lowering=False, debug=False,
                   num_devices=8)
    P = {}

    def dparam(name, shape, dtype=F32):
        P[name] = nc.declare_dram_parameter(name, list(shape), dtype,
                                            isOutput=False)

    dparam("xin", [ND, 128, T])
    dparam("mask", [1, T])
    dparam("statw", [128, 1], BF16)
    dparam("ones_row", [1, 128], BF16)
    dparam("ident", [128, 128], BF16)
    dparam("gate", [128, 1])
    dparam("ln0w", [128, ND]); dparam("ln0b", [128, ND])
    for l in range(N_LAYERS):
        dparam(f"w_in{l}", [ND, 128, 1536], BF16)
        dparam(f"cw{l}", [128, NE * D_CONV])
        dparam(f"convb{l}", [128, NE])
        dparam(f"w_xp{l}", [NE, 128, 64], BF16)
        dparam(f"w_dt{l}", [32, HALF_E], BF16)
        dparam(f"ndtb{l}", [128, NEH])                # NEGATED dt_proj_b
        dparam(f"negA{l}", [128, NEH * D_STATE])      # -A[e,n] (positive)
        dparam(f"Ddiag{l}", [NEH, 128, 128], BF16)
        dparam(f"w_out{l}", [NEH, 128, D_MODEL], BF16)
        dparam(f"w_f1{l}", [ND, 128, HALF_F_PAD], BF16)
        dparam(f"b1{l}", [128, ND])
        dparam(f"w_f2{l}", [ND, 128, D_MODEL], BF16)
        dparam(f"b2g{l}", [128, ND])
        for ln in ("ln1", "ln2"):
            dparam(f"{ln}w{l}", [128, ND])
            dparam(f"{ln}b{l}", [128, ND])
    out_ext = nc.declare_dram_parameter("out", [ND, 128, T], F32,
                                        isOutput=True)

    with tile.TileContext(nc) as tc:
        with (
            tc.tile_pool(name="wc", bufs=1) as wc,        # constants
            tc.tile_pool(name="wl", bufs=1) as wl,
            tc.tile_pool(name="wcv", bufs=8) as wcv,        # per-layer weights
            tc.tile_pool(name="wi", bufs=1) as wi,        # in_proj (prefetch)
            tc.tile_pool(name="act", bufs=1) as act,
            tc.tile_pool(name="sm", bufs=1) as sm,        # small [1,T] rows
            tc.tile_pool(name="scan", bufs=4) as scanp,
            tc.tile_pool(name="bc", bufs=1) as bcp,
            tc.tile_pool(name="pp", bufs=3, space="PSUM") as pp,
            tc.tile_pool(name="pd", bufs=1, space="PSUM") as pd,
            tc.tile_pool(name="py", bufs=2, space="PSUM") as py,
            tc.tile_pool(name="pst", bufs=1, space="PSUM") as pst,
            tc.tile_pool(name="dram", bufs=2, space="DRAM") as dram,
        ):
            # ---- constants ----
            def cload(name, pool=wc, tag=None):
                shp = list(P[name].shape)
                t_ = pool.tile(shp, P[name].dtype, tag=tag or name)
                nc.sync.dma_start(t_[:], P[name][:])
                return t_

            # critical-path loads first: residual input + LN0 params
            x_cur = []
            for j in range(ND):
                xt = act.tile([128, T], F32, tag=f"tmp{j}", name=f"xin{j}")
                nc.sync.dma_start(xt[:], P["xin"][j])
                x_cur.append(xt)
            statw = cload("statw")
            ones_row = cload("ones_row")
            ident = cload("ident")
            gate = cload("gate")
            ln0w = cload("ln0w"); ln0b = cload("ln0b")
            lnp = {}
            for l in range(N_LAYERS):
                for ln in ("ln1", "ln2"):
                    lnp[f"{ln}w{l}"] = cload(f"{ln}w{l}")
                    lnp[f"{ln}b{l}"] = cload(f"{ln}b{l}")
            smallp = {}
            for l in range(N_LAYERS):
                for nm in (f"convb{l}", f"ndtb{l}", f"negA{l}", f"b1{l}",
                           f"b2g{l}", f"cw{l}"):
                    smallp[nm] = cload(nm)

            maskrep = wc.tile([128, T], F32, tag="maskrep")
            nc.sync.dma_start(
                maskrep[:], P["mask"][:].partition_broadcast(128)[:, 0, :])

            xm_pad = nc.alloc_sbuf_tensor("xm_pad", [128, NE * (T + 3)], BF16)
            nc.vector.memset(xm_pad.ap()[:], 0.0)

            def layernorm(x_tiles, w_col, b_col, out_dtype, out_tags):
                xb, x2 = [], []
                for j in range(ND):
                    b_ = act.tile([128, T], BF16, tag=f"lnxb{j}")
                    s_ = act.tile([128, T], BF16, tag=f"lnx2{j}")
                    nc.vector.tensor_copy(b_[:], x_tiles[j][:])
                    nc.scalar.activation(s_[:], x_tiles[j][:], AF.Square)
                    xb.append(b_); x2.append(s_)
                mu_ps = pst.tile([1, T], F32, tag="mu")
                e2_ps = pst.tile([1, T], F32, tag="e2")
                for j in range(ND):
                    nc.tensor.matmul(mu_ps[:], statw[:, 0:1], xb[j][:],
                                     start=(j == 0), stop=(j == ND - 1))
                for j in range(ND):
                    nc.tensor.matmul(e2_ps[:], statw[:, 0:1], x2[j][:],
                                     start=(j == 0), stop=(j == ND - 1))
                mus = sm.tile([1, T], F32, tag="mus")
                mub = sm.tile([1, T], BF16, tag="mub")
                e2s = sm.tile([1, T], F32, tag="e2s")
                nc.scalar.activation(mus[:], mu_ps[:], AF.Copy)
                nc.scalar.activation(mub[:], mu_ps[:], AF.Copy)
                nc.scalar.activation(e2s[:], e2_ps[:], AF.Copy)
                mu2 = sm.tile([1, T], F32, tag="mu2")
                var = sm.tile([1, T], F32, tag="var")
                sd = sm.tile([1, T], F32, tag="sd")
                rb = sm.tile([1, T], BF16, tag="rb")
                nc.vector.tensor_tensor(mu2[:], mus[:], mus[:], OP.mult)
                nc.vector.tensor_tensor(var[:], e2s[:], mu2[:], OP.subtract)
                nc.vector.tensor_scalar_add(var[:], var[:], EPS)
                nc.scalar.activation(sd[:], var[:], AF.Sqrt)
                with nc.allow_low_precision(reason="bf16 rstd is enough"):
                    nc.vector.reciprocal(rb[:], sd[:])
                murep = act.tile([128, T], BF16, tag="murep")
                rrep = act.tile([128, T], BF16, tag="rrep")
                mu_rep_ps = pp.tile([128, T], F32, tag="mm")
                nc.tensor.matmul(mu_rep_ps[:], ones_row[:], mub[:],
                                 start=True, stop=True)
                nc.scalar.activation(murep[:], mu_rep_ps[:], AF.Copy)
                r_rep_ps = pp.tile([128, T], F32, tag="mm")
                nc.tensor.matmul(r_rep_ps[:], ones_row[:], rb[:],
                                 start=True, stop=True)
                nc.scalar.activation(rrep[:], r_rep_ps[:], AF.Copy)
                outs = []
                for j in range(ND):
                    xh = act.tile([128, T], BF16, tag=f"lnxh{j}")
                    nc.vector.tensor_tensor(xh[:], xb[j][:], murep[:],
                                            OP.subtract)
                    nc.vector.tensor_tensor(xh[:], xh[:], rrep[:], OP.mult)
                    o = act.tile([128, T], out_dtype, tag=out_tags.format(j=j))
                    nc.scalar.activation(o[:], xh[:], AF.Identity,
                                         scale=w_col[:, j:j + 1],
                                         bias=b_col[:, j:j + 1])
                    outs.append(o)
                return outs

            x_cur = layernorm(x_cur, ln0w, ln0b, F32, "xr{j}")

            for l in range(N_LAYERS):
                # per-layer weight loads (shared tags -> reused slots)
                w_in = []
                for k in range(ND):
                    t_ = wi.tile([128, 1536], BF16, tag=f"w_in{k}")
                    nc.sync.dma_start(t_[:], P[f"w_in{l}"][k])
                    w_in.append(t_)

                w_xp = []
                for k in range(NE):
                    t_ = wl.tile([128, 64], BF16, tag=f"w_xp{k}")
                    nc.sync.dma_start(t_[:], P[f"w_xp{l}"][k])
                    w_xp.append(t_)
                w_dt = wl.tile([32, HALF_E], BF16, tag="w_dt")
                nc.sync.dma_start(w_dt[:], P[f"w_dt{l}"][:])
                Ddiag = []
                for k in range(NEH):
                    t_ = wl.tile([128, 128], BF16, tag=f"Ddiag{k}")
                    nc.sync.dma_start(t_[:], P[f"Ddiag{l}"][k])
                    Ddiag.append(t_)
                w_out = []
                for k in range(NEH):
                    t_ = wl.tile([128, D_MODEL], BF16, tag=f"w_out{k}")
                    nc.sync.dma_start(t_[:], P[f"w_out{l}"][k])
                    w_out.append(t_)
                w_f1 = []
                for k in range(ND):
                    t_ = wl.tile([128, HALF_F_PAD], BF16, tag=f"w_f1{k}")
                    nc.sync.dma_start(t_[:], P[f"w_f1{l}"][k])
                    w_f1.append(t_)
                w_f2 = []
                for k in range(ND):
                    t_ = wl.tile([128, D_MODEL], BF16, tag=f"w_f2{k}")
                    nc.sync.dma_start(t_[:], P[f"w_f2{l}"][k])
                    w_f2.append(t_)

                xh = layernorm(x_cur, lnp[f"ln1w{l}"], lnp[f"ln1b{l}"], BF16,
                               "lno{j}")

                # ---- in_proj ----
                zs = []
                for m in range(12):
                    ps = pp.tile([128, T], F32, tag="mm")
                    for k in range(ND):
                        nc.tensor.matmul(ps[:],
                                         w_in[k][:, m * 128:(m + 1) * 128],
                                         xh[k][:], start=(k == 0),
                                         stop=(k == ND - 1))
                    if m < 8:
                        off = m * (T + 3) + 3
                        nc.scalar.activation(xm_pad.ap()[:, off:off + T],
                                             ps[:], AF.Copy)
                    else:
                        z = act.tile([128, T], BF16, tag=f"z{m - 8}")
                        nc.scalar.activation(z[:], ps[:], AF.Silu)
                        zs.append(z)

                # ---- conv (DVE taps) + silu -> u ----
                u = []
                for j in range(NE):
                    base = j * (T + 3)
                    cacc = act.tile([128, T], BF16, tag="cacc")
                    ct = [scanp.tile([128, T], BF16, tag="ctap",
                                     name=f"ct{k}") for k in range(D_CONV)]
                    for k in range(D_CONV):
                        nc.vector.tensor_scalar_mul(
                            ct[k][:], xm_pad.ap()[:, base + k:base + k + T],
                            smallp[f"cw{l}"][:, j * D_CONV + k:
                                             j * D_CONV + k + 1])
                    nc.vector.tensor_tensor(cacc[:], ct[0][:], ct[1][:],
                                            OP.add)
                    nc.vector.tensor_tensor(cacc[:], cacc[:], ct[2][:],
                                            OP.add)
                    nc.vector.tensor_tensor(cacc[:], cacc[:], ct[3][:],
                                            OP.add)
                    uj = act.tile([128, T], BF16, tag=f"u{j}")
                    nc.scalar.activation(uj[:], cacc[:], AF.Silu,
                                         bias=smallp[f"convb{l}"][:, j:j + 1])
                    u.append(uj)

                # ---- x_proj ----
                dbl_ps = pd.tile([64, T], F32, tag="dbl")
                for k in range(NE):
                    nc.tensor.matmul(dbl_ps[:], w_xp[k][:], u[k][:],
                                     start=(k == 0), stop=(k == NE - 1))
                dbl = act.tile([64, T], BF16, tag="dbl_sb")
                nc.scalar.activation(dbl[:], dbl_ps[:], AF.Copy)

                # ---- B / C replication ----
                bcd = dram.tile([32, T], BF16, tag="bcd")
                nc.sync.dma_start(bcd[:], dbl[32:64, :])
                Brep, Crep = [], []
                for n in range(D_STATE):
                    br = bcp.tile([128, T], BF16, tag=f"Brep{n}")
                    cr = bcp.tile([128, T], BF16, tag=f"Crep{n}")
                    nc.sync.dma_start(
                        br[:], bcd[n:n + 1, :]
                        .partition_broadcast(128)[:, 0, :])
                    nc.sync.dma_start(
                        cr[:], bcd[16 + n:17 + n, :]
                        .partition_broadcast(128)[:, 0, :])
                    Brep.append(br); Crep.append(cr)

                # ---- delta for all m (batched per ACT table) ----
                sgs, dms, wdus = [], [], []
                for m in range(NEH):
                    ps = pp.tile([128, T], F32, tag="mm")
                    nc.tensor.matmul(ps[:],
                                     w_dt[0:32, m * 128:(m + 1) * 128],
                                     dbl[0:32, :], start=True, stop=True)
                    sg = act.tile([128, T], BF16, tag=f"sg{m}")
                    nc.scalar.activation(sg[:], ps[:], AF.Sigmoid, scale=-1.0,
                                         bias=smallp[f"ndtb{l}"][:, m:m + 1])
                    sgs.append(sg)
                for m in range(NEH):
                    dm = act.tile([128, T], BF16, tag=f"dm{m}")
                    nc.scalar.activation(dm[:], sgs[m][:], AF.Ln)
                    dms.append(dm)
                    # wdu = delta' * u = -softplus*u; sign fixed via negated
                    # C rows (host-side) since y is linear in dBu.
                    wdu = act.tile([128, T], BF16, tag=f"wdu{m}")
                    nc.vector.tensor_tensor(wdu[:], dm[:], u[m][:], OP.mult)
                    wdus.append(wdu)

                # warm up ncfw so the out_proj AllReduce pays less first-call
                # latency: tiny dummy collective issued early in the scan
                warm_in = dram.tile([1, 4], F32, tag="warm_in")
                warm_out = dram.tile([1, 4], F32, tag="warm_out")
                nc.sync.dma_start(warm_in[:], maskrep[0:1, 0:4])
                nc.gpsimd.collective_compute(
                    "AllReduce", OP.add, replica_groups=RG,
                    ins=[warm_in[:].opt()], outs=[warm_out[:].opt()])

                # ---- scan (per own e-tile) ----
                y_gated = []
                for m in range(NEH):
                    dm = dms[m]
                    wdu = wdus[m]
                    ps_y = py.tile([128, T], F32, tag="ps_y")
                    for n in range(D_STATE):
                        dA = scanp.tile([128, T], BF16, tag="dA")
                        nc.scalar.activation(
                            dA[:], dm[:], AF.Exp,
                            scale=smallp[f"negA{l}"]
                            [:, m * D_STATE + n: m * D_STATE + n + 1])
                        dBu = scanp.tile([128, T], BF16, tag="dBu")
                        nc.vector.tensor_tensor(dBu[:], wdu[:], Brep[n][:],
                                                OP.mult)
                        hs = scanp.tile([128, T], BF16, tag="hs")
                        nc.vector.tensor_tensor_scan(hs[:], dA[:], dBu[:],
                                                     0.0, OP.mult, OP.add)
                        hsC = scanp.tile([128, T], BF16, tag="hsC")
                        nc.vector.tensor_tensor(hsC[:], hs[:], Crep[n][:],
                                                OP.mult)
                        nc.tensor.matmul(ps_y[:], ident[:], hsC[:],
                                         start=(n == 0), stop=False)
                    nc.tensor.matmul(ps_y[:], Ddiag[m][:], u[m][:],
                                     start=False, stop=True)
                    yg = act.tile([128, T], BF16, tag=f"yg{m}")
                    nc.vector.tensor_tensor(yg[:], ps_y[:], zs[m][:], OP.mult)
                    y_gated.append(yg)

                # ---- out_proj partial + AllReduce ----
                arA_in = dram.tile([128, ND * T], BF16, tag="arA_in")
                arA_out = dram.tile([128, ND * T], BF16, tag="arA_out")
                poA = act.tile([128, ND * T], BF16, tag="stageA")
                for g in range(2):
                    pso = [pp.tile([128, T], F32, tag="mm", name=f"pso{j}")
                           for j in (2 * g, 2 * g + 1)]
                    for k in range(NEH):
                        for i, j in enumerate((2 * g, 2 * g + 1)):
                            nc.tensor.matmul(
                                pso[i][:], w_out[k][:, j * 128:(j + 1) * 128],
                                y_gated[k][:], start=(k == 0),
                                stop=(k == NEH - 1))
                    for i, j in enumerate((2 * g, 2 * g + 1)):
                        nc.scalar.activation(poA[:, j * T:(j + 1) * T],
                                             pso[i][:], AF.Copy)
                nc.sync.dma_start(arA_in[:], poA[:])
                nc.gpsimd.collective_compute(
                    "AllReduce", OP.add, replica_groups=RG,
                    ins=[arA_in[:].opt()], outs=[arA_out[:].opt()])
                moA = act.tile([128, ND * T], BF16, tag="stageAo")
                nc.sync.dma_start(moA[:], arA_out[:])
                x_mid = []
                for j in range(ND):
                    xm_ = act.tile([128, T], F32, tag=f"xmid{j}")
                    nc.vector.tensor_tensor(xm_[:], x_cur[j][:],
                                            moA[:, j * T:(j + 1) * T], OP.add)
                    x_mid.append(xm_)

                # ---- FFN ----
                xh2 = layernorm(x_mid, lnp[f"ln2w{l}"], lnp[f"ln2b{l}"], BF16,
                                "lno{j}")
                h = []
                for m in range(ND):
                    ps = pp.tile([128, T], F32, tag="mm")
                    for k in range(ND):
                        nc.tensor.matmul(ps[:],
                                         w_f1[k][:, m * 128:(m + 1) * 128],
                                         xh2[k][:], start=(k == 0),
                                         stop=(k == ND - 1))
                    hm = act.tile([128, T], BF16, tag=f"h{m}")
                    nc.scalar.activation(hm[:], ps[:], AF.Gelu,
                                         bias=smallp[f"b1{l}"][:, m:m + 1])
                    h.append(hm)

                if l < N_LAYERS - 1:
                    arB_in = dram.tile([128, ND * T], BF16, tag="arB_in")
                    arB_out = dram.tile([128, ND * T], BF16, tag="arB_out")
                    poB = act.tile([128, ND * T], BF16, tag="stageA")
                    for g in range(2):
                        psf = [pp.tile([128, T], F32, tag="mm",
                                       name=f"psf{j}")
                               for j in (2 * g, 2 * g + 1)]
                        for k in range(ND):
                            for i, j in enumerate((2 * g, 2 * g + 1)):
                                nc.tensor.matmul(
                                    psf[i][:],
                                    w_f2[k][:, j * 128:(j + 1) * 128],
                                    h[k][:], start=(k == 0),
                                    stop=(k == ND - 1))
                        for i, j in enumerate((2 * g, 2 * g + 1)):
                            nc.scalar.activation(
                                poB[:, j * T:(j + 1) * T], psf[i][:], AF.Copy)
                    nc.sync.dma_start(arB_in[:], poB[:])
                    nc.gpsimd.collective_compute(
                        "AllReduce", OP.add, replica_groups=RG,
                        ins=[arB_in[:].opt()], outs=[arB_out[:].opt()])
                    moB = act.tile([128, ND * T], BF16, tag="stageAo")
                    nc.sync.dma_start(moB[:], arB_out[:])
                    x_next = []
                    for j in range(ND):
                        t1 = act.tile([128, T], F32, tag=f"tmp{j}")
                        nc.vector.scalar_tensor_tensor(
                            t1[:], moB[:, j * T:(j + 1) * T],
                            smallp[f"b2g{l}"][:, j:j + 1],
                            x_mid[j][:], OP.add, OP.add)
                        xn = act.tile([128, T], F32, tag=f"xr{j}")
                        nc.vector.tensor_tensor(xn[:], t1[:], maskrep[:],
                                                OP.mult)
                        x_next.append(xn)
                    x_cur = x_next
                else:
                    for j in range(ND):
                        ps = pp.tile([128, T], F32, tag="mm")
                        for k in range(ND):
                            nc.tensor.matmul(
                                ps[:], w_f2[k][:, j * 128:(j + 1) * 128],
                                h[k][:], start=(k == 0), stop=(k == ND - 1))
                        t1 = act.tile([128, T], F32, tag=f"tmp{j}")
                        nc.vector.scalar_tensor_tensor(
                            t1[:], x_mid[j][:], gate[:, 0:1], ps[:],
                            OP.mult, OP.add)
                        t2 = act.tile([128, T], F32, tag=f"cpy{j}")
                        nc.vector.scalar_tensor_tensor(
                            t2[:], t1[:], smallp[f"b2g{l}"][:, j:j + 1],
                            maskrep[:], OP.add, OP.mult)
                        nc.sync.dma_start(out_ext[j], t2[:])

    nc.compile()
    return nc


def _sinusoidal_pe(Tn, D):
    pos = np.arange(Tn, dtype=np.float32)[:, None]
    div = np.exp(np.arange(0, D, 2, dtype=np.float32) *
                 (-np.log(10000.0) / D))
    pe = np.zeros((Tn, D), dtype=np.float32)
    pe[:, 0::2] = np.sin(pos * div)
    pe[:, 1::2] = np.cos(pos * div)
    return pe


def _col128(v, ncols):
    v = np.asarray(v, np.float32)
    if v.shape[0] < ncols * 128:
        v = np.pad(v, (0, ncols * 128 - v.shape[0]))
    return v.reshape(ncols, 128).T.copy()


def _prep_core_inputs(x, lengths, params):
    import ml_dtypes
    x = np.asarray(x, np.float32)
    lengths = np.asarray(lengths)
    pe = _sinusoidal_pe(T, D_MODEL)

    def f(a):
        return np.ascontiguousarray(np.asarray(a, np.float32))

    def bf(a):
        return np.ascontiguousarray(
            np.asarray(a, np.float32).astype(ml_dtypes.bfloat16))

    in_maps = []
    for c in range(8):
        s, p = c // 2, c % 2
        b, d = s // 2, s % 2
        prm = [params["fwd"], params["bwd"]][d]
        L = int(lengths[b])

        xi = (x[b] + pe).astype(np.float32)
        if d == 1:
            xi = xi[::-1]
        mask = (np.arange(T) < L).astype(np.float32)
        if d == 1:
            mask = mask[::-1]

        e_own = np.arange(p * HALF_E, (p + 1) * HALF_E)
        e_oth = np.arange((1 - p) * HALF_E, (2 - p) * HALF_E)
        e_ord = np.concatenate([e_own, e_oth])
        f_own = np.arange(p * HALF_F, (p + 1) * HALF_F)

        m = {
            "xin": f(xi.T.reshape(ND, 128, T)),
            "mask": f(mask[None, :]),
            "statw": bf(np.full((128, 1), 1.0 / D_MODEL)),
            "ones_row": bf(np.ones((1, 128))),
            "ident": bf(np.eye(128)),
            "gate": f(np.full((128, 1), 1.0 if p == 0 else 0.0)),
            "ln0w": f(_col128(prm[0]["ln0_w"], ND)),
            "ln0b": f(_col128(prm[0]["ln0_b"], ND)),
        }
        for l in range(N_LAYERS):
            lp = prm[l]
            ip = np.asarray(lp["in_proj_w"], np.float32)
            w_in = np.concatenate([ip[e_ord], ip[D_INNER + e_own]], 0).T
            m[f"w_in{l}"] = bf(np.ascontiguousarray(w_in).reshape(ND, 128,
                                                                  1536))
            cw = np.asarray(lp["conv_w"], np.float32)[e_ord]
            cwcols = np.zeros((128, NE * D_CONV), np.float32)
            for j in range(NE):
                for k in range(D_CONV):
                    cwcols[:, j * D_CONV + k] = cw[j * 128:(j + 1) * 128, k]
            m[f"cw{l}"] = f(cwcols)
            m[f"convb{l}"] = f(_col128(
                np.asarray(lp["conv_b"], np.float32)[e_ord], NE))
            xp = np.asarray(lp["x_proj_w"], np.float32).copy()
            xp[DT_RANK + D_STATE:] *= -1.0    # negate C rows (sign fold)
            m[f"w_xp{l}"] = bf(np.ascontiguousarray(xp[:, e_ord].T)
                               .reshape(NE, 128, 64))
            dtw = np.asarray(lp["dt_proj_w"], np.float32)
            m[f"w_dt{l}"] = bf(dtw[e_own].T)
            m[f"ndtb{l}"] = f(_col128(
                -np.asarray(lp["dt_proj_b"], np.float32)[e_own], NEH))
            A = -np.exp(np.asarray(lp["A_log"], np.float32))
            negA = -A[e_own]
            m[f"negA{l}"] = f(np.concatenate(
                [negA[mm * 128:(mm + 1) * 128] for mm in range(NEH)], axis=1))
            Dv = np.asarray(lp["D"], np.float32)[e_own]
            dd = np.zeros((NEH, 128, 128), np.float32)
            for mm in range(NEH):
                np.fill_diagonal(dd[mm], Dv[mm * 128:(mm + 1) * 128])
            m[f"Ddiag{l}"] = bf(dd)
            ow = np.asarray(lp["out_proj_w"], np.float32)
            m[f"w_out{l}"] = bf(np.ascontiguousarray(ow[:, e_own].T)
                                .reshape(NEH, 128, D_MODEL))
            f1 = np.asarray(lp["ffn_w1"], np.float32)
            f1p = np.zeros((HALF_F_PAD, D_MODEL), np.float32)
            f1p[:HALF_F] = f1[f_own]
            m[f"w_f1{l}"] = bf(np.ascontiguousarray(f1p.T)
                               .reshape(ND, 128, HALF_F_PAD))
            b1p = np.zeros(HALF_F_PAD, np.float32)
            b1p[:HALF_F] = np.asarray(lp["ffn_b1"], np.float32)[f_own]
            m[f"b1{l}"] = f(_col128(b1p, ND))
            f2 = np.asarray(lp["ffn_w2"], np.float32)
            f2p = np.zeros((HALF_F_PAD, D_MODEL), np.float32)
            f2p[:HALF_F] = f2[:, f_own].T
            m[f"w_f2{l}"] = bf(f2p.reshape(ND, 128, D_MODEL))
            b2 = np.asarray(lp["ffn_b2"], np.float32)
            m[f"b2g{l}"] = f(_col128(b2 if p == 0 else np.zeros_like(b2), ND))
            for ln, wkey, bkey in (("ln1", "ln1_w", "ln1_b"),
                                   ("ln2", "ln2_w", "ln2_b")):
                m[f"{ln}w{l}"] = f(_col128(lp[wkey], ND))
                m[f"{ln}b{l}"] = f(_col128(lp[bkey], ND))
        in_maps.append(m)
    return in_maps


def kernel(x, lengths, params):
    if "nc" not in _CACHED:
        _CACHED["nc"] = _build()
    nc = _CACHED["nc"]
    in_maps = _prep_core_inputs(x, lengths, params)
    res = None
    for attempt in range(3):
        try:
            res = run_bass_kernel_spmd(nc, in_maps, list(range(8)))
            break
        except Exception:
            if attempt == 2:
                raise
            import time
            time.sleep(10)
    outs = [np.asarray(res.results[c]["out"], np.float32)
            .reshape(D_MODEL, T) for c in range(8)]
    final = np.zeros((B, T, D_MODEL), np.float32)
    for b in range(B):
        fwd = (outs[4 * b + 0] + outs[4 * b + 1]).T
        bwd = (outs[4 * b + 2] + outs[4 * b + 3]).T[::-1]
        final[b] = fwd + bwd
    return final
